# revision 5
# baseline (speedup 1.0000x reference)
"""Trainium2 Bass kernel for nn_DictlessHeteroLayer (hetero GNN message passing).

  out = sum_r [ x @ W_self[r].T + b_self[r]
                + scatter_add_dst( ew * (x @ W_nei[r].T)[src] ) ]

Strategy (8 NeuronCores, SPMD, no collectives):
  * Linearity: scatter_add(ew * (x@W_r^T)[src]) == scatter_add(ew * x[src]) @ W_r^T,
    so aggregate RAW x rows per (dst tile, relation) first, then apply W_r once
    per 128-row dst tile.  This removes the baseline's phase-1 H=x@W^T HBM
    round-trip (~128 MB/core) entirely.
  * Host assigns dst nodes to 128-slot tiles (degree balanced, first-fit
    decreasing), deals tiles to cores (edge balanced).  Each core fully owns
    its tiles' output rows -> no cross-core reduction; host re-assembles.
  * Host builds a per-core EDGE-ALIGNED, SBUF-LAYOUT message table
    Mt[p, chunk, d] = fp16 x[src(slot)], slot = chunk*128+p, in schedule
    order.  The device streams it with plain contiguous DMA (8 KB/partition
    runs, full 360 GB/s) -- no dma_gather (2x descriptor penalty), no idx
    tables, GPSIMD freed.
  * Per 128-edge chunk and (tile, rel) cell: an engine builds a one-hot
    OH[e, dst_slot] = (iota==dst)*ew in fp16 (DVE / GPSIMD / Act, tunable
    split); TensorE accumulates A_r^T[d, dst] += sum_e g[e, d]*OH[e, dst]
    into the tile's PSUM bank quarter (bank = tile, quarter = relation).
  * Stage 2 per tile: copy bank -> SBUF fp16, then 4+1 matmuls
    out[dst, d] = sum_r A_r @ W_r^T + x_tile @ (sum_r W_self)^T accumulate
    in-place into quarter 0 of the same bank; bias added on host.
  * Duplicate (rel, src, dst) edges merged on host (weights summed, exact).
"""
import numpy as np

import concourse.bacc as bacc
import concourse.bass as bass
import concourse.mybir as mybir
import concourse.tile as tile
from concourse import bass_utils

P = 128
D = 128
NC = 8
import os as _os
MAX_CALL_CHUNKS = int(_os.environ.get("KMAXCALL", "64"))
GBUFS = int(_os.environ.get("KGBUFS", "4"))
OHBUFS = int(_os.environ.get("KOHBUFS", "16"))
ABUFS = int(_os.environ.get("KABUFS", "8"))
SPBUFS = int(_os.environ.get("KSPBUFS", "2"))
WAVE = 8                     # dst tiles per PSUM wave (1 bank per tile)
# one-hot engine weights (per-op cost ~ DVE:94ns, Pool:273ns, Act:2x238ns)
OH_DVE = int(_os.environ.get("KOH_DVE", "11"))
OH_POOL = int(_os.environ.get("KOH_POOL", "4"))
OH_ACT = int(_os.environ.get("KOH_ACT", "0"))


# ----------------------------------------------------------------- scheduling
class Sched:
    pass


def build_schedule(inputs):
    import ml_dtypes
    x = np.asarray(inputs["x"], np.float32)
    ei = np.asarray(inputs["edge_index"])
    ew = np.asarray(inputs["edge_weight"], np.float32)
    rel_ptr = np.asarray(inputs["rel_ptr"]).astype(np.int64)
    W_self = np.asarray(inputs["W_self"], np.float32)
    b_self = np.asarray(inputs["b_self"], np.float32)
    W_nei = np.asarray(inputs["W_nei"], np.float32)

    N = x.shape[0]
    E = ei.shape[1]
    NREL = W_nei.shape[0]
    NT0 = -(-N // P)
    T_CORE = -(-NT0 // NC)
    NT = T_CORE * NC

    src = ei[0].astype(np.int64)
    dst = ei[1].astype(np.int64)
    rel = (np.searchsorted(rel_ptr, np.arange(E), side="right") - 1).astype(np.int64)

    # merge duplicate (rel, src, dst) edges (sum their weights) -- exact
    ukey = (rel * N + src) * N + dst
    uorder = np.argsort(ukey, kind="stable")
    uk = ukey[uorder]
    first = np.ones(E, bool)
    first[1:] = uk[1:] != uk[:-1]
    gids = np.cumsum(first) - 1
    ew_sum = np.zeros(int(gids[-1]) + 1, np.float64)
    np.add.at(ew_sum, gids, ew[uorder].astype(np.float64))
    keep = uorder[first]
    src, dst, rel = src[keep], dst[keep], rel[keep]
    ew = ew_sum.astype(np.float32)
    E = len(src)

    deg = np.bincount(dst, minlength=N)

    # ---- node -> (tile, slot): first-fit decreasing over NT tiles
    import heapq
    order = np.argsort(-deg, kind="stable")
    tile_of = np.empty(N, np.int64)
    slot_of = np.empty(N, np.int64)
    heap = [(0, t, 0) for t in range(NT)]
    heapq.heapify(heap)
    for n in order:
        load, t, used = heapq.heappop(heap)
        tile_of[n] = t
        slot_of[n] = used
        used += 1
        if used < P:
            heapq.heappush(heap, (load + int(deg[n]), t, used))

    tile_load = np.bincount(tile_of[dst], minlength=NT)

    # ---- tiles -> cores (greedy balance); local index = per-core fill order
    # (descending global load => local j pairs similar-load tiles across
    # cores, minimizing the shared-schedule max-over-core cell padding)
    t_order = np.argsort(-tile_load, kind="stable")
    core_of_tile = np.empty(NT, np.int64)
    local_of_tile = np.empty(NT, np.int64)
    heap = [(0, c, 0) for c in range(NC)]
    heapq.heapify(heap)
    core_fill = [0] * NC
    for t in t_order:
        load, c, cnt_ = heapq.heappop(heap)
        core_of_tile[t] = c
        local_of_tile[t] = core_fill[c]
        core_fill[c] += 1
        if core_fill[c] < T_CORE:
            heapq.heappush(heap, (load + int(tile_load[t]), c, core_fill[c]))

    # ---- per-edge attributes
    e_tile = tile_of[dst]
    e_core = core_of_tile[e_tile]
    e_j = local_of_tile[e_tile]              # local tile 0..T_CORE-1
    e_r = rel

    # ---- waves
    wave_sizes = []
    j = 0
    while j < T_CORE:
        wave_sizes.append(min(WAVE, T_CORE - j))
        j += WAVE
    NW = len(wave_sizes)

    # ---- static cell table: seg_len[j, r] = max over cores of edge count
    cnt = np.zeros((NC, T_CORE, NREL), np.int64)
    np.add.at(cnt, (e_core, e_j, e_r), 1)
    seg_len = cnt.max(axis=0)                # [T_CORE, NREL]

    # ---- slot layout: per wave, cells (j, r) packed contiguously; wave
    # segment padded to x128.  chunks = global 128-slot windows.
    cell_off = np.zeros((T_CORE, NREL), np.int64)
    off = 0
    npair = 0
    pair_meta = []        # (pair, chunk_slot0, j, r, lo, hi)
    wave_plans = []       # per wave: list of calls; call = [chunk -> [(pair, j, r, stop)]]
    wave_info = []
    w0 = 0
    for w, wsz in enumerate(wave_sizes):
        jlo, jhi = w0, w0 + wsz
        seg0 = off
        ranges = []
        for j in range(jlo, jhi):
            for r in range(NREL):
                if seg_len[j, r] == 0:
                    continue
                cell_off[j, r] = off
                ranges.append((j, r, off, off + seg_len[j, r]))
                off += seg_len[j, r]
        seg_edges = off - seg0
        nch = -(-seg_edges // P) if seg_edges else 0
        off = seg0 + nch * P                  # pad wave segment to x128
        chunk_list = []
        for k in range(nch):
            c0, c1 = seg0 + k * P, seg0 + (k + 1) * P
            mms = []
            for (j, r, lo, hi) in ranges:
                if hi <= c0 or lo >= c1:
                    continue
                mms.append([npair, j, r, False])
                pair_meta.append((npair, c0, j, r, max(lo, c0), min(hi, c1)))
                npair += 1
            chunk_list.append(mms)
        # stop flag: last pair per bank j (emission order = chunk asc)
        last_of_bank = {}
        first_of_bank = {}
        for mms in chunk_list:
            for ent in mms:
                jj = ent[1]
                if jj not in first_of_bank:
                    first_of_bank[jj] = ent[0]
                last_of_bank[jj] = ent
        for ent in last_of_bank.values():
            ent[3] = True
        calls = []
        pos = 0
        while pos < len(chunk_list):
            n = min(MAX_CALL_CHUNKS, len(chunk_list) - pos)
            calls.append((seg0 // P + pos, chunk_list[pos: pos + n]))
            pos += n
        wave_plans.append(calls)
        wave_info.append(dict(
            w=w, wsz=wsz, jlo=jlo, first_pair=set(first_of_bank.values()),
            rels=[[r for r in range(NREL) if seg_len[j, r] > 0]
                  for j in range(jlo, jhi)],
        ))
        w0 += wsz
    NPAIR = npair
    total_slots = off
    CH_TOTAL = total_slots // P

    # ---- per-core flat edge arrays in schedule order
    key = (e_core * T_CORE + e_j) * NREL + e_r
    sort_idx = np.lexsort((src, key))
    skey = key[sort_idx]
    newg = np.ones(E, bool)
    newg[1:] = skey[1:] != skey[:-1]
    group_first = np.nonzero(newg)[0]
    group_id = np.cumsum(newg) - 1
    rank = np.arange(E) - group_first[group_id]

    se = sort_idx
    pos_in_core = cell_off[e_j[se], e_r[se]] + rank
    core_se = e_core[se]

    src_flat = np.zeros((NC, total_slots), np.int64)
    dst_flat = np.zeros((NC, total_slots), np.float32)
    ew_flat = np.zeros((NC, total_slots), np.float32)
    src_flat[core_se, pos_in_core] = src[se]
    dst_flat[core_se, pos_in_core] = slot_of[dst[se]].astype(np.float32)
    ew_flat[core_se, pos_in_core] = ew[se]

    # ---- edge-aligned message table, SBUF layout: Mt[c][p, chunk*D + d]
    x16 = x.astype(np.float16)
    mt = []
    for c in range(NC):
        m = x16[src_flat[c]]                          # [slots, D] fp16
        m[ew_flat[c] == 0.0] = 0
        mt.append(np.ascontiguousarray(
            m.reshape(CH_TOTAL, P, D).transpose(1, 0, 2).reshape(P, CH_TOTAL * D)))

    # ---- masked per-(chunk, cell) pair columns [NC, 128, NPAIR] fp16
    dst_dev = np.zeros((NC, P, NPAIR), np.float32)
    ew_dev = np.zeros((NC, P, NPAIR), np.float32)
    for (pr, c0, j, r, lo, hi) in pair_meta:
        a, bnd = lo - c0, hi - c0
        dst_dev[:, a:bnd, pr] = dst_flat[:, lo:hi]
        ew_dev[:, a:bnd, pr] = ew_flat[:, lo:hi]

    # ---- dense inputs (all fp16)
    WT4 = np.empty((D, NREL * D), np.float16)
    for r in range(NREL):
        WT4[:, r * D: (r + 1) * D] = W_nei[r].T.astype(np.float16)
    WselfT = W_self.sum(axis=0).T.astype(np.float16).copy()   # [k, d]
    bsum = b_self.sum(axis=0).astype(np.float32)
    iotaf = np.tile(np.arange(P, dtype=np.float16), (P, 1))

    # xtp per core: [NC, 128, T_CORE*128] column (j*128+p) = x[node(j,p)]
    node_at = np.full((NC, T_CORE, P), -1, np.int64)
    node_at[core_of_tile[tile_of], local_of_tile[tile_of], slot_of] = np.arange(N)
    xtp = np.zeros((NC, D, T_CORE * P), np.float16)
    for c in range(NC):
        nn = node_at[c].reshape(-1)
        valid = nn >= 0
        xtp[c][:, valid] = x16[nn[valid]].T

    s = Sched()
    s.N, s.E, s.NT, s.T_CORE, s.NW, s.NREL = N, E, NT, T_CORE, NW, NREL
    s.wave_sizes = wave_sizes
    s.wave_plans = wave_plans
    s.wave_info = wave_info
    s.CH_TOTAL = CH_TOTAL
    s.NPAIR = NPAIR
    s.seg_len = seg_len
    s.core_of_tile, s.local_of_tile = core_of_tile, local_of_tile
    s.tile_of, s.slot_of = tile_of, slot_of
    s.in_shared = dict(wt4=WT4, wselft=WselfT, iotaf=iotaf)
    s.bsum = bsum
    s.in_percore = [
        dict(mt=mt[c], dstc=dst_dev[c], ewc=ew_dev[c], xtp=xtp[c])
        for c in range(NC)
    ]
    return s


# ----------------------------------------------------------------- bass build
def build_bass(s, num_devices=NC):
    f16 = mybir.dt.float16
    f32 = mybir.dt.float32
    NREL = s.NREL

    nc = bacc.Bacc("TRN2", num_devices=num_devices)
    mt = nc.dram_tensor("mt", [P, s.CH_TOTAL * D], f16, kind="ExternalInput")
    wt4 = nc.dram_tensor("wt4", [P, NREL * D], f16, kind="ExternalInput")
    wselft = nc.dram_tensor("wselft", [P, D], f16, kind="ExternalInput")
    iotaf = nc.dram_tensor("iotaf", [P, P], f16, kind="ExternalInput")
    xtp = nc.dram_tensor("xtp", [P, s.T_CORE * P], f16, kind="ExternalInput")
    dstc = nc.dram_tensor("dstc", [P, s.NPAIR], f32, kind="ExternalInput")
    ewc = nc.dram_tensor("ewc", [P, s.NPAIR], f32, kind="ExternalInput")
    outF = nc.dram_tensor("outF", [P, s.T_CORE * D], f16, kind="ExternalOutput")

    # one-hot engine rotation
    rot = [0] * OH_DVE + [1] * OH_POOL + [2] * OH_ACT
    if not rot:
        rot = [0]

    with tile.TileContext(nc) as tc:
        with (
            tc.tile_pool(name="const", bufs=1) as cpool,
            tc.tile_pool(name="meta", bufs=2) as mpool,
            tc.tile_pool(name="g", bufs=GBUFS) as gpool,
            tc.tile_pool(name="oh", bufs=OHBUFS) as ohpool,
            tc.tile_pool(name="a", bufs=ABUFS) as apool,
            tc.tile_pool(name="st", bufs=SPBUFS) as spool,
            tc.tile_pool(name="p2", bufs=1, space="PSUM") as p2pool,
        ):
            wt4_t = cpool.tile([P, NREL * D], f16)
            nc.sync.dma_start(out=wt4_t[:], in_=wt4[:, :])
            wselft_t = cpool.tile([P, D], f16)
            nc.sync.dma_start(out=wselft_t[:], in_=wselft[:, :])
            iota_t = cpool.tile([P, P], f16)
            nc.sync.dma_start(out=iota_t[:], in_=iotaf[:, :])
            dst_t = cpool.tile([P, s.NPAIR], f32)
            nc.sync.dma_start(out=dst_t[:], in_=dstc[:, :])
            ew_t = cpool.tile([P, s.NPAIR], f32)
            nc.sync.dma_start(out=ew_t[:], in_=ewc[:, :])
            xp_t = cpool.tile([P, s.T_CORE * P], f16)
            nc.sync.dma_start(out=xp_t[:], in_=xtp[:, :])

            oh_ctr = [0]

            def build_oh(oh, pr):
                eng = rot[oh_ctr[0] % len(rot)]
                oh_ctr[0] += 1
                if eng == 1:
                    nc.gpsimd.tensor_scalar(
                        out=oh[:], in0=iota_t[:],
                        scalar1=dst_t[:, pr: pr + 1],
                        scalar2=ew_t[:, pr: pr + 1],
                        op0=mybir.AluOpType.is_equal,
                        op1=mybir.AluOpType.mult,
                    )
                else:
                    nc.vector.tensor_scalar(
                        out=oh[:], in0=iota_t[:],
                        scalar1=dst_t[:, pr: pr + 1],
                        scalar2=ew_t[:, pr: pr + 1],
                        op0=mybir.AluOpType.is_equal,
                        op1=mybir.AluOpType.mult,
                    )

            for wi, calls in zip(s.wave_info, s.wave_plans):
                w, wsz, jlo = wi["w"], wi["wsz"], wi["jlo"]
                first_pair = wi["first_pair"]
                banks = [
                    p2pool.tile([P, NREL * P], f32, space="PSUM",
                                tag=f"bank{k}", name=f"bank{k}_w{w}")
                    for k in range(wsz)
                ]

                # ---------------- stage 1: chunk streams + one-hot matmuls
                for (ch0, chunk_list) in calls:
                    nch_ = len(chunk_list)
                    g_t = gpool.tile([P, nch_ * D], f16, tag="g")
                    nc.sync.dma_start(
                        out=g_t[:], in_=mt[:, ch0 * D: (ch0 + nch_) * D]
                    )
                    for pos, mms in enumerate(chunk_list):
                        for (pr, j, r, stop) in mms:
                            oh = ohpool.tile([P, P], f16, tag="oh")
                            build_oh(oh, pr)
                            nc.tensor.matmul(
                                out=banks[j - jlo][:, r * P: (r + 1) * P],
                                lhsT=g_t[:, pos * D: (pos + 1) * D],
                                rhs=oh[:],
                                start=(pr in first_pair),
                                stop=stop,
                                skip_group_check=True,
                            )

                # ---------------- stage 2: per tile, A_r @ W_r^T + self
                stage = spool.tile([P, wsz, P], f16, tag="stage")
                for j in range(jlo, jlo + wsz):
                    jj = j - jlo
                    rels = wi["rels"][jj]
                    bank = banks[jj]
                    if rels:
                        a_sb = apool.tile([P, NREL * P], f16, tag="a")
                        for r in rels:
                            nc.scalar.copy(
                                out=a_sb[:, r * P: (r + 1) * P],
                                in_=bank[:, r * P: (r + 1) * P])
                    nmm = len(rels) + 1
                    for i, r in enumerate(rels):
                        nc.tensor.matmul(
                            out=bank[:, 0:P],
                            lhsT=a_sb[:, r * P: (r + 1) * P],
                            rhs=wt4_t[:, r * P: (r + 1) * P],
                            start=(i == 0),
                            stop=False,
                            skip_group_check=True,
                        )
                    nc.tensor.matmul(
                        out=bank[:, 0:P],
                        lhsT=xp_t[:, j * P: (j + 1) * P],
                        rhs=wselft_t[:],
                        start=(len(rels) == 0),
                        stop=True,
                        skip_group_check=True,
                    )
                    nc.scalar.copy(out=stage[:, jj, :], in_=bank[:, 0:P])
                dview = outF[:, jlo * D: (jlo + wsz) * D]
                nc.scalar.dma_start(out=dview, in_=stage[:].rearrange("p t d -> p (t d)"))
    nc.compile()
    return nc


def kernel(**inputs):
    s = build_schedule(inputs)
    nc = build_bass(s)
    in_maps = []
    for c in range(NC):
        m = dict(s.in_shared)
        m.update(s.in_percore[c])
        in_maps.append(m)
    res = bass_utils.run_bass_kernel_spmd(nc, in_maps, core_ids=list(range(NC)))
    outF = np.stack([res.results[c]["outF"] for c in range(NC)])  # [NC,P,T*D]
    return assemble(s, outF)


def assemble(s, outF):
    N = s.N
    out = outF.reshape(NC, P, s.T_CORE, D).astype(np.float32)
    nodes = np.arange(N)
    c = s.core_of_tile[s.tile_of[nodes]]
    t = s.local_of_tile[s.tile_of[nodes]]
    p = s.slot_of[nodes]
    return out[c, p, t, :] + s.bsum[None, :]
